# revision 16
# baseline (speedup 1.0000x reference)
"""MultiHeadAttention (B=2, S=2048, D=1024, H=16, causal) on 8 trn2 NeuronCores.

Sharding: tensor-parallel over heads (2 heads/core) for QKV projections and
attention; two AllToAlls (one per batch) re-shard context rows so the output
projection is data-parallel over rows; bias added on device. Host only
slices/transposes/casts inputs and reassembles outputs.

Per-core output rows: global rows [c*256,(c+1)*256) (batch 0 part) and
[2048+c*256, 2048+(c+1)*256) (batch 1 part) — the per-batch A2A split lets
batch-0's collective and output projection overlap batch-1's attention.

Math notes:
  - torch-Linear semantics: q = x @ Wq.T etc. Host passes transposed weight
    shards so all matmuls contract over the SBUF partition dim.
  - softmax without max-subtraction (scores*inv_scale is O(1), exp is safe);
    denominator comes from a ones-column appended to V in the attn@V matmul.
  - normalization is deferred and batched per batch: denominators are copied
    out, reciprocal'd in one 8-lane op, broadcast across partitions with a
    selector matmul (f32r), and multiplied in (keeps PE unstalled).
  - reference quirk preserved: scale = 1/(D**0.25).
"""

import math
import sys
import types

import numpy as np
import ml_dtypes

N_CORES = 8
B, S, D = 2, 2048, 1024
H = 16
HEAD = 64
ROWS = B * S               # 4096
ROWS_PER_CORE = ROWS // N_CORES  # 512
INV_SCALE = 1.0 / (D ** 0.25)
QT = 512                   # query tile (free dim)
KT = 128                   # key tile (partition dim)
N_QT = S // QT             # 4 query tiles per batch
RH = 128                   # rows per core per A2A segment

BF16 = ml_dtypes.bfloat16

_compiled = None


def _install_axon_profile_shim():
    """Provide antenv.axon_hooks (missing from this image) so trace=True works,
    and neuter the artifact upload (no bucket access in-container)."""
    if "antenv.axon_hooks" not in sys.modules:
        mod = types.ModuleType("antenv.axon_hooks")
        mod._hook = None
        mod.set_axon_ntff_profile_hook = lambda h: setattr(mod, "_hook", h)
        mod.get_axon_ntff_profile_hook = lambda: mod._hook
        sys.modules["antenv.axon_hooks"] = mod
        try:
            import antenv
            antenv.axon_hooks = mod
        except ImportError:
            pass
    mod = sys.modules["antenv.axon_hooks"]
    if mod._hook is None:
        try:
            from trn_agent_boot.trn_boot import _ntff_profile_via_ctypes
            mod.set_axon_ntff_profile_hook(
                _ntff_profile_via_ctypes("/opt/axon/libaxon_pjrt.so"))
        except Exception:
            pass
    try:
        import concourse.bass_utils as bu
        bu.upload_artifacts = lambda tmpdir: tmpdir
    except Exception:
        pass


def _split_excess_waits(nc, max_waits=1):
    """walrus in this container only encodes one sem-wait per instruction;
    hoist extras onto InstEventSemaphore instructions inserted just before."""
    import concourse.mybir as mybir
    n = 0
    for fn in nc.m.functions:
        for bb in fn.blocks:
            out = []
            for inst in bb.instructions:
                si = inst.sync_info
                if si is not None and si.on_wait and len(si.on_wait) > max_waits:
                    waits = list(si.on_wait)
                    excess, keep = waits[:-max_waits], waits[-max_waits:]
                    for i in range(0, len(excess), max_waits):
                        ev = mybir.InstEventSemaphore(
                            name=f"{inst.name}-wsplit{n}",
                            engine=inst.engine,
                            ins=[], outs=[],
                            sync_info=mybir.SyncInfo(
                                on_wait=list(excess[i:i + max_waits]),
                                on_update=[]),
                        )
                        n += 1
                        out.append(ev)
                    si.on_wait = keep
                out.append(inst)
            bb.instructions = out
    return n


def _build_program(debug=False):
    import concourse.bass as bass
    import concourse.mybir as mybir
    import concourse.tile as tile
    from concourse.bass import ts

    f32 = mybir.dt.float32
    f32r = mybir.dt.float32r
    bf16 = mybir.dt.bfloat16
    Exp = mybir.ActivationFunctionType.Exp

    nc = bass.Bass(num_devices=N_CORES)
    dbg = {}
    if debug:
        dbg["qT"] = nc.dram_tensor("dbg_qT", [128, ROWS], bf16,
                                   kind="ExternalOutput")
        dbg["kT"] = nc.dram_tensor("dbg_kT", [128, ROWS], bf16,
                                   kind="ExternalOutput")
        dbg["v"] = nc.dram_tensor("dbg_v", [128, 32, 2, HEAD + 1], bf16,
                                  kind="ExternalOutput")
        dbg["ctx0"] = nc.dram_tensor("dbg_ctx0", [64, ROWS], bf16,
                                     kind="ExternalOutput")
        dbg["ctx1"] = nc.dram_tensor("dbg_ctx1", [64, ROWS], bf16,
                                     kind="ExternalOutput")

    xT = nc.dram_tensor("xT", [D, ROWS], bf16, kind="ExternalInput")
    wqT = nc.dram_tensor("wqT", [128, 8, 128], bf16, kind="ExternalInput")
    wkT = nc.dram_tensor("wkT", [128, 8, 128], bf16, kind="ExternalInput")
    wvT = nc.dram_tensor("wvT", [128, 8, 128], bf16, kind="ExternalInput")
    woT = nc.dram_tensor("woT", [128, 8, D], bf16, kind="ExternalInput")
    bo = nc.dram_tensor("bo", [D], f32, kind="ExternalInput")
    masksq = nc.dram_tensor("masksq", [128, 128], bf16, kind="ExternalInput")
    sel = nc.dram_tensor("sel", [4, 4 * HEAD], f32r, kind="ExternalInput")
    out_rows = nc.dram_tensor("out_rows", [ROWS_PER_CORE, D], f32,
                              kind="ExternalOutput")

    with tile.TileContext(nc) as tc:
        with (
            tc.tile_pool(name="persist", bufs=1) as persist,
            tc.tile_pool(name="cp", bufs=4) as cp,
            tc.tile_pool(name="attn", bufs=6) as attn_pool,
            tc.tile_pool(name="nrm", bufs=4) as nrm_pool,
            tc.tile_pool(name="ps_work", bufs=3, space="PSUM") as ps_work,
            tc.tile_pool(name="ps_scores", bufs=2, space="PSUM") as ps_scores,
            tc.tile_pool(name="ps_bc", bufs=1, space="PSUM") as ps_bc,
            tc.tile_pool(name="dram", bufs=1, space="DRAM") as dram,
        ):
            # ---- persistent SBUF state ----
            xT_sb = persist.tile([128, 8, ROWS], bf16)        # 64 KB/part
            wq_sb = persist.tile([128, 8, 128], bf16)
            wk_sb = persist.tile([128, 8, 128], bf16)
            wv_sb = persist.tile([128, 8, 128], bf16)
            woT_sb = persist.tile([128, 8, D], bf16)          # 16 KB/part
            qT_sb = persist.tile([128, ROWS], bf16)           # 8 KB/part
            kT_sb = persist.tile([128, ROWS], bf16)
            vT_sb = persist.tile([128, ROWS], bf16)
            # one tile per (rowtile, head): xbar-transpose needs offset-0
            # contiguous dst; col 64 is the ones column for the denominator
            v_tiles = [[persist.tile([128, HEAD + 1], bf16, tag=f"v{rt}_{h}",
                                     name=f"v{rt}_{h}")
                        for h in range(2)] for rt in range(32)]
            ctx_sb = [persist.tile([64, ROWS], bf16, tag=f"ctx{h}",
                                   name=f"ctx{h}")
                      for h in range(2)]
            mask_sb = persist.tile([128, 128], bf16)
            sel_sb = persist.tile([4, 4 * HEAD], f32r)
            den_all = [persist.tile([4, QT], f32, tag=f"den{g}",
                                    name=f"den{g}") for g in range(4)]
            den_rec = [persist.tile([4, QT], f32r, tag=f"rec{g}",
                                    name=f"rec{g}") for g in range(4)]
            bo_sb = persist.tile([128, D], f32)
            a2a_sb = [persist.tile([128, 8, RH], bf16, tag=f"a2a{g}",
                                   name=f"a2a{g}") for g in range(4)]

            # ---- loads: small weights first, xT split across both queues ----
            nc.sync.dma_start(wv_sb[:], wvT[:])
            nc.scalar.dma_start(wq_sb[:], wqT[:])
            nc.scalar.dma_start(wk_sb[:], wkT[:])
            for kt in range(8):
                eng = nc.sync if kt % 2 == 0 else nc.scalar
                eng.dma_start(xT_sb[:, kt, :], xT[ts(kt, 128), :])
            nc.gpsimd.dma_start(woT_sb[:], woT[:])
            nc.gpsimd.dma_start(mask_sb[:], masksq[:])
            nc.gpsimd.dma_start(sel_sb[:], sel[:])
            nc.gpsimd.dma_start(
                bo_sb[:], bass.AP(tensor=bo, offset=0,
                                  ap=[[0, 128], [1, D]]))
            for rt in range(32):
                for h in range(2):
                    nc.gpsimd.memset(v_tiles[rt][h][:, HEAD:HEAD + 1], 1.0)

            # ---- projections: vT first so transposes overlap q/k ----
            tp_i = 0
            for w_sb, dst in ((wv_sb, vT_sb), (wq_sb, qT_sb), (wk_sb, kT_sb)):
                for rt in range(8):
                    ps = ps_work.tile([128, 512], f32, tag="work")
                    for kt in range(8):
                        nc.tensor.matmul(ps, w_sb[:, kt, :],
                                         xT_sb[:, kt, ts(rt, 512)],
                                         start=(kt == 0), stop=(kt == 7))
                    nc.vector.tensor_copy(dst[:, ts(rt, 512)], ps)
                    if dst is vT_sb:
                        # the 4 row-tiles of 128 this 512-chunk covers
                        for rt128 in range(rt * 4, rt * 4 + 4):
                            for h in range(2):
                                eng = nc.sync if tp_i % 2 == 0 else nc.scalar
                                eng.dma_start_transpose(
                                    v_tiles[rt128][h][:, 0:HEAD],
                                    vT_sb[h * HEAD:(h + 1) * HEAD,
                                          ts(rt128, 128)])
                                tp_i += 1

            def attention_half(b, half):
                """Attention for query tiles qt in {2*half, 2*half+1}."""
                units = []
                for qt in (2 * half, 2 * half + 1):
                    q0 = b * S + qt * QT
                    n_k = 4 * qt + 4
                    ps_av = [ps_work.tile([HEAD + 1, QT], f32, tag="work",
                                          name=f"av{b}_{qt}_{h}")
                             for h in range(2)]
                    for jk in range(n_k):
                        o = jk - 4 * qt       # >=0 on the diagonal band
                        c0 = max(o, 0) * 128  # first live query column
                        k0 = b * S + jk * KT
                        ps_s = ps_scores.tile([128, 2, QT], f32, tag="sc")
                        at = attn_pool.tile([128, 2, QT], bf16, tag="at")
                        for h in range(2):
                            hs = slice(h * HEAD, (h + 1) * HEAD)
                            nc.tensor.matmul(
                                ps_s[:, h, c0:QT],
                                kT_sb[hs, k0:k0 + KT],
                                qT_sb[hs, q0 + c0:q0 + QT],
                                start=True, stop=True)
                        nc.scalar.activation(at[:, :, c0:QT], ps_s[:, :, c0:QT],
                                             Exp, scale=INV_SCALE)
                        if o >= 0:
                            # partial causal sub-block: cols [c0, c0+128)
                            nc.vector.tensor_mul(
                                at[:, :, c0:c0 + 128],
                                at[:, :, c0:c0 + 128],
                                mask_sb[:, None, :].to_broadcast([128, 2, 128]))
                        for h in range(2):
                            nc.tensor.matmul(
                                ps_av[h][:, c0:QT],
                                v_tiles[b * 16 + jk][h][:],
                                at[:, h, c0:QT],
                                start=(jk == 0), stop=(jk == n_k - 1))
                    g = b * 2 + half
                    for h in range(2):
                        u = (qt - 2 * half) * 2 + h
                        nc.vector.tensor_copy(ctx_sb[h][:, q0:q0 + QT],
                                              ps_av[h][0:HEAD, :])
                        dtmp = nrm_pool.tile([HEAD + 1, QT], f32, tag="dtmp")
                        nc.vector.tensor_copy(dtmp[HEAD:HEAD + 1, :],
                                              ps_av[h][HEAD:HEAD + 1, :])
                        nc.sync.dma_start(den_all[g][u:u + 1, :],
                                          dtmp[HEAD:HEAD + 1, :])
                        units.append((h, q0, u))
                return units

            def norm_and_a2a(b, half, units):
                g = b * 2 + half
                with nc.allow_low_precision(
                        reason="softmax denominators: f32r keeps ~19 mantissa "
                               "bits, ample for a 1/x broadcast"):
                    nc.vector.reciprocal(den_rec[g][:], den_all[g][:])
                for (h, q0, u) in units:
                    ps_b = ps_bc.tile([HEAD, QT], f32, tag="bc")
                    nc.tensor.matmul(ps_b,
                                     sel_sb[:, ts(u, HEAD)],
                                     den_rec[g][:],
                                     start=True, stop=True)
                    nc.vector.tensor_mul(ctx_sb[h][:, q0:q0 + QT],
                                         ctx_sb[h][:, q0:q0 + QT],
                                         ps_b[:])
                # A2A for segment g: shard s = rows [b*2048+half*1024+s*128,+128)
                r0 = b * S + half * (S // 2)
                a2a_in = dram.tile([8, 128, RH], bf16, tag=f"a2ain{g}",
                                   name=f"a2ain{g}")
                a2a_out = dram.tile([8, 128, RH], bf16, tag=f"a2aout{g}",
                                    name=f"a2aout{g}")
                for h in range(2):
                    nc.sync.dma_start(
                        a2a_in[:, h * 64:(h + 1) * 64, :]
                        .rearrange("s p r -> p s r"),
                        ctx_sb[h][:, r0:r0 + S // 2]
                        .rearrange("p (s r) -> p s r", s=8))
                nc.gpsimd.collective_compute(
                    "AllToAll", mybir.AluOpType.bypass,
                    replica_groups=[list(range(N_CORES))],
                    ins=[a2a_in[:].opt()], outs=[a2a_out[:].opt()])
                nc.scalar.dma_start(a2a_sb[g][:],
                                    a2a_out[:].rearrange("t p r -> p t r"))

            def outproj(g):
                # segment g rows land in out_rows[g*128:(g+1)*128]
                for nh in range(2):
                    ps = ps_work.tile([128, 512], f32, tag="work")
                    for t in range(8):
                        nc.tensor.matmul(ps,
                                         a2a_sb[g][:, t, :],
                                         woT_sb[:, t, ts(nh, 512)],
                                         start=(t == 0), stop=(t == 7))
                    ob = cp.tile([128, 512], f32, tag="ob")
                    nc.vector.tensor_add(ob, ps, bo_sb[:, ts(nh, 512)])
                    nc.sync.dma_start(
                        out_rows[ts(g, 128), ts(nh, 512)], ob)

            u00 = attention_half(0, 0)
            norm_and_a2a(0, 0, u00)
            u01 = attention_half(0, 1)
            norm_and_a2a(0, 1, u01)
            u10 = attention_half(1, 0)
            outproj(0)
            norm_and_a2a(1, 0, u10)
            u11 = attention_half(1, 1)
            outproj(1)
            norm_and_a2a(1, 1, u11)
            outproj(2)
            outproj(3)

            if debug:
                nc.sync.dma_start(dbg["qT"][:], qT_sb[:])
                nc.sync.dma_start(dbg["kT"][:], kT_sb[:])
                for rt in range(32):
                    for h in range(2):
                        nc.sync.dma_start(dbg["v"][:, rt, h, :],
                                          v_tiles[rt][h][:])
                nc.sync.dma_start(dbg["ctx0"][:], ctx_sb[0][:])
                nc.sync.dma_start(dbg["ctx1"][:], ctx_sb[1][:])

    _split_excess_waits(nc)
    return nc


def _make_masksq():
    p = np.arange(128)[:, None]
    r = np.arange(128)[None, :]
    return (p <= r).astype(BF16)


def _make_sel():
    # sel[k, u*64+m] = 1 if k == u : broadcasts den_rec row u over 64 partitions
    s = np.zeros((4, 4 * HEAD), np.float32)
    for u in range(4):
        s[u, u * HEAD:(u + 1) * HEAD] = 1.0
    return s


def _wlayout(wT):
    # [1024, m] -> [128, 8, m] with dst[p, t, :] = wT[t*128+p, :]
    m = wT.shape[1]
    return np.ascontiguousarray(
        wT.reshape(8, 128, m).transpose(1, 0, 2)).astype(BF16)


def _shard_inputs(x, Wq, Wk, Wv, Wo, bo):
    xT = np.ascontiguousarray(
        x.reshape(ROWS, D).T).astype(BF16)            # [D, 4096]
    woT = _wlayout(Wo.T)                              # [128, 8, D]
    masksq = _make_masksq()
    sel = _make_sel()
    bo32 = np.ascontiguousarray(bo.astype(np.float32))
    maps = []
    for c in range(N_CORES):
        rs = slice(c * 128, (c + 1) * 128)
        maps.append({
            "xT": xT,
            "wqT": _wlayout(Wq[rs].T),
            "wkT": _wlayout(Wk[rs].T),
            "wvT": _wlayout(Wv[rs].T),
            "woT": woT,
            "bo": bo32,
            "masksq": masksq,
            "sel": sel,
        })
    return maps


def kernel(x, Wq, Wk, Wv, Wo, bo, trace=False):
    global _compiled
    _install_axon_profile_shim()
    from concourse.bass_utils import run_bass_kernel_spmd

    x = np.asarray(x, dtype=np.float32)
    Wq = np.asarray(Wq, dtype=np.float32)
    Wk = np.asarray(Wk, dtype=np.float32)
    Wv = np.asarray(Wv, dtype=np.float32)
    Wo = np.asarray(Wo, dtype=np.float32)
    bo = np.asarray(bo, dtype=np.float32)

    if _compiled is None:
        _compiled = _build_program()
    nc = _compiled

    in_maps = _shard_inputs(x, Wq, Wk, Wv, Wo, bo)
    res = run_bass_kernel_spmd(nc, in_maps, core_ids=list(range(N_CORES)),
                               trace=trace)
    out = np.empty((ROWS, D), np.float32)
    for c in range(N_CORES):
        r = res.results[c]["out_rows"]
        for g in range(4):
            b, half = g // 2, g % 2
            r0 = b * S + half * (S // 2) + c * RH
            out[r0:r0 + RH] = r[g * RH:(g + 1) * RH]
    out = out.reshape(B, S, D)
    if trace:
        kernel.last_exec_time_ns = res.exec_time_ns
        kernel.last_results = res
    return out
